# revision 51
# baseline (speedup 1.0000x reference)
"""Trainium2 Bass kernel for AdditiveVisioLinguistic attention.

reference:
    att1 = enc @ W_enc + b_enc              # [B, P, A]
    att2 = dec @ W_dec + b_dec              # [B, A]
    att  = relu(att1 + att2[:, None]) @ W_full + b_full   # [B, P]
    alpha = softmax(att, axis=1)
    awe  = einsum('bpe,bp->be', enc, alpha)
    return (awe, alpha)

Sharding: pure data parallel over the batch dim (16 batches per core, 8
cores), weights replicated.  b_full is dropped on-device (softmax is shift
invariant); b_enc+b_dec are merged host-side; W_enc/W_dec/W_full/dec are
pre-cast to bf16 host-side (the big matmuls run in bf16, PSUM accumulates
fp32; worst-case rel err ~2.5e-3).

Per-core: rows = 16*196 = 3136 flattened (b,p) rows, processed in
r-tile-aligned chunks (a small 128-row first chunk absorbs the DMA ramp,
then 512-row chunks).  The E=2048 contraction needs enc with E on
partitions: enc tiles are cast to bf16 and PE-transposed (bf16 transposes
may write bf16 PSUM), then copied to SBUF; att1 accumulates over 16
K-chunks into 4 PSUM banks; add-bias+ReLU is fused into one ScalarE
activation per batch segment (bias = att2T column); att comes from a
W_full matvec as [1, rows].

awe is fused into the main loop ("flash" style): per chunk, the raw att
rows are PE-transposed onto partitions, w = exp(att - 8) is computed on
128 lanes and masked into block-diagonal weight tiles (host-provided
mask), and V += wblk.T @ enc runs on the still-resident bf16 enc tiles.
Since the final normalization awe = V / sum(exp(att - 8)) is exactly
shift-invariant, a fixed shift of 8 replaces the running max (att for
this data is O(1), so exp stays well inside fp32 range).  alpha uses a
true max-subtracted softmax on a [16, 196] layout gathered incrementally
through a DRAM bounce (cross-partition reshapes are DMA-only on TRN2).
"""

import numpy as np

N_CORES = 8
B_GLOB = 128
B_LOC = B_GLOB // N_CORES     # 16
P_PIX = 196
E_DIM = 2048
A_DIM = 512
D_DIM = 512

ROWS = B_LOC * P_PIX          # 3136
RC = 512                      # rows per main chunk
N_RC = (ROWS + RC - 1) // RC  # 7 (last chunk has 64 valid rows)
KE = E_DIM // 128             # 16
KD = D_DIM // 128             # 4
NA = A_DIM // 128             # 4
RT = (ROWS + 127) // 128      # 25 r-tiles (last has 64 valid rows)
ESHIFT = 8.0                  # fixed exp shift for the fused awe pass

_CACHE = {}


def _batch_segments(r0, r1):
    """Batch segments overlapping rows [r0, r1): list of (batch, c0, c1)
    where [c0, c1) are column offsets relative to r0."""
    segs = []
    b0 = r0 // P_PIX
    b1 = (min(r1, ROWS) - 1) // P_PIX
    for b in range(b0, b1 + 1):
        s0 = max(r0, b * P_PIX)
        s1 = min(min(r1, ROWS), (b + 1) * P_PIX)
        if s1 > s0:
            segs.append((b, s0 - r0, s1 - r0))
    return segs


def _build(debug=False):
    import concourse.bacc as bacc
    import concourse.mybir as mybir
    import concourse.tile as tile
    from concourse.masks import make_identity
    from contextlib import ExitStack

    F32 = mybir.dt.float32
    F32R = mybir.dt.float32r
    BF16 = mybir.dt.bfloat16
    Relu = mybir.ActivationFunctionType.Relu
    Exp = mybir.ActivationFunctionType.Exp
    AX = mybir.AxisListType.X

    def r_(ap):
        return ap.bitcast(F32R)

    nc = bacc.Bacc(None, target_bir_lowering=False)

    enc = nc.dram_tensor("enc", [ROWS, E_DIM], F32, kind="ExternalInput")
    dec = nc.dram_tensor("dec", [B_LOC, D_DIM], BF16, kind="ExternalInput")
    w_enc = nc.dram_tensor("w_enc", [E_DIM, A_DIM], BF16, kind="ExternalInput")
    w_dec = nc.dram_tensor("w_dec", [D_DIM, A_DIM], BF16, kind="ExternalInput")
    bias_a = nc.dram_tensor("bias_a", [A_DIM], F32, kind="ExternalInput")
    w_full = nc.dram_tensor("w_full", [A_DIM], BF16, kind="ExternalInput")
    maskt = nc.dram_tensor("maskt", [128, RT, B_LOC], F32, kind="ExternalInput")
    att_dram = nc.dram_tensor("att_bounce", [ROWS], F32)
    out_awe = nc.dram_tensor("out_awe", [B_LOC, E_DIM], F32, kind="ExternalOutput")
    out_alpha = nc.dram_tensor(
        "out_alpha", [B_LOC, P_PIX], F32, kind="ExternalOutput"
    )
    if debug:
        dbg_att = nc.dram_tensor("dbg_att", [1, ROWS], F32, kind="ExternalOutput")
        dbg_v = nc.dram_tensor("dbg_v", [B_LOC, E_DIM], F32, kind="ExternalOutput")

    with tile.TileContext(nc) as tc, ExitStack() as ctx:
        consts = ctx.enter_context(tc.tile_pool(name="consts", bufs=1))

        ident_f = consts.tile([128, 128], F32)
        make_identity(nc, ident_f[:])
        ident = consts.tile([128, 128], F32)
        nc.vector.tensor_copy(r_(ident[:]), ident_f[:])
        ident_bf = consts.tile([128, 128], BF16)
        nc.vector.tensor_copy(ident_bf[:], ident_f[:])

        # warm the PE clock gate during the initial DMA wait: dummy
        # transposes of the identity (last one overwrites ident_bf with
        # ident_bf.T == identity, so the chain is not dead code)
        with tc.tile_pool(name="psWarm", bufs=2, space="PSUM") as psWarm:
            wp = None
            for _ in range(36):
                wp = psWarm.tile([128, 128], BF16, tag="warm", name="wp")
                nc.tensor.transpose(wp[:], ident_bf[:], ident_bf[:])
            nc.vector.tensor_copy(ident_bf[:], wp[:])

        dec_sb = consts.tile([B_LOC, D_DIM], BF16)
        nc.sync.dma_start(dec_sb[:], dec[:])
        wdec_sb = consts.tile([128, KD, A_DIM], BF16)
        nc.sync.dma_start(
            wdec_sb[:], w_dec[:].rearrange("(ko ki) a -> ki ko a", ki=128)
        )
        bias_sb = consts.tile([128, NA], F32)
        nc.sync.dma_start(bias_sb[:], bias_a[:].rearrange("(o p) -> p o", p=128))
        wfull_sb = consts.tile([128, NA], BF16)
        nc.sync.dma_start(wfull_sb[:], w_full[:].rearrange("(o p) -> p o", p=128))


        att_all = consts.tile([1, ROWS], F32)
        att_bp = consts.tile([B_LOC, P_PIX], F32)
        neg8 = consts.tile([128, 1], F32)
        nc.vector.memset(neg8[:], -ESHIFT)
        att2T = consts.tile([128, NA, B_LOC], F32)
        vsum = consts.tile([B_LOC, E_DIM], F32)  # unnormalized awe accumulator
        wbp = []
        for i in range(4):
            wt = consts.tile([128, 128], BF16, name=f"wbp{i}")
            nc.vector.memset(wt[:], 0.0)
            wbp.append(wt)

        # ---------------- Phase A: att2T = (dec @ W_dec + bias).T ----------
        with tc.tile_pool(name="psA", bufs=2, space="PSUM") as psA:
            decT = consts.tile([128, KD, B_LOC], BF16)
            for kc in range(KD):
                pt = psA.tile([128, B_LOC], BF16, tag="psA")
                nc.tensor.transpose(
                    pt[:],
                    dec_sb[:, kc * 128 : (kc + 1) * 128],
                    ident_bf[:B_LOC, :B_LOC],
                )
                nc.vector.tensor_copy(decT[:, kc, :], pt[:])
            for a in range(NA):
                pa = psA.tile([128, B_LOC], F32, tag="psA2")
                for kc in range(KD):
                    nc.tensor.matmul(
                        pa[:],
                        wdec_sb[:, kc, a * 128 : (a + 1) * 128],
                        decT[:, kc, :],
                        start=(kc == 0),
                        stop=(kc == KD - 1),
                    )
                nc.vector.tensor_scalar_add(att2T[:, a, :], pa[:], bias_sb[:, a : a + 1])

        # ------------- Phase B: att1, relu, att, fused awe accumulate ------
        with (
            tc.tile_pool(name="nat", bufs=5) as nat_pool,
            tc.tile_pool(name="natbf", bufs=9) as natbf_pool,
            tc.tile_pool(name="encT", bufs=3) as encT_pool,
            tc.tile_pool(name="relu", bufs=2) as relu_pool,
            tc.tile_pool(name="wexp", bufs=2) as wexp_pool,
            tc.tile_pool(name="wblk", bufs=8) as wblk_pool,
            tc.tile_pool(name="psM", bufs=4, space="PSUM") as psM,
            tc.tile_pool(name="psT", bufs=3, space="PSUM") as psT,
            tc.tile_pool(name="psAtt", bufs=1, space="PSUM") as psAtt,
        ):
            # prefetch the first chunk's enc tiles ahead of the big weight DMA
            nats_cache = {}
            for i, t in enumerate(range(0, 4)):
                h = min(128, ROWS - 128 * t)
                nt = nat_pool.tile([128, E_DIM], F32, tag="nat", name=f"pre{t}")
                nc.sync.dma_start(r_(nt[:h, :]), r_(enc[128 * t : 128 * t + h, :]))
                nb = natbf_pool.tile([128, E_DIM], BF16, tag="natbf", name=f"prebf{t}")
                if i % 2 == 0:
                    nc.vector.tensor_copy(nb[:h, :], nt[:h, :])
                else:
                    nc.scalar.copy(nb[:h, :], nt[:h, :])
                nats_cache[t] = nb
            maskt_sb = consts.tile([128, RT, B_LOC], F32)
            nc.sync.dma_start(maskt_sb[:], maskt[:])
            wenc_sb = consts.tile([128, KE, A_DIM], BF16)
            nc.sync.dma_start(
                wenc_sb[:], w_enc[:].rearrange("(ko ki) a -> ki ko a", ki=128)
            )
            bounds = [0, 128, 640, 1152, 1664, 2176, 2688, ROWS]
            gathered = 0
            for rc in range(len(bounds) - 1):
                r0 = bounds[rc]
                r1 = bounds[rc + 1]
                w = r1 - r0
                tiles = []  # (t, h, col0)
                for t in range(r0 // 128, (r1 + 127) // 128):
                    h = min(128, ROWS - 128 * t)
                    tiles.append((t, h, 128 * t - r0))

                nats = {}
                for i, (t, h, _) in enumerate(tiles):
                    if t in nats_cache:
                        nats[t] = nats_cache.pop(t)
                        continue
                    nt = nat_pool.tile([128, E_DIM], F32, tag="nat")
                    nc.sync.dma_start(r_(nt[:h, :]), r_(enc[128 * t : 128 * t + h, :]))
                    nb = natbf_pool.tile([128, E_DIM], BF16, tag="natbf", name=f"nb{t}")
                    if i % 2 == 0:
                        nc.vector.tensor_copy(nb[:h, :], nt[:h, :])
                    else:
                        nc.scalar.copy(nb[:h, :], nt[:h, :])
                    nats[t] = nb

                enT = encT_pool.tile([128, KE, RC], BF16, tag="encT")
                for k in range(KE):
                    pt = psT.tile([128, RC], BF16, tag="psT")
                    for t, h, c0 in tiles:
                        nc.tensor.transpose(
                            pt[:, c0 : c0 + h],
                            nats[t][:h, k * 128 : (k + 1) * 128],
                            ident_bf[:h, :h],
                        )
                    # alternate copy engine to balance DVE/ACT load
                    if k % 2 == 0:
                        nc.vector.tensor_copy(enT[:, k, :w], pt[:, :w])
                    else:
                        nc.scalar.copy(enT[:, k, :w], pt[:, :w])

                pm = [psM.tile([128, RC], F32, tag="psM", name=f"pm{a}") for a in range(NA)]
                for k in range(KE):
                    for a in range(NA):
                        nc.tensor.matmul(
                            pm[a][:, :w],
                            wenc_sb[:, k, a * 128 : (a + 1) * 128],
                            enT[:, k, :w],
                            start=(k == 0),
                            stop=(k == KE - 1),
                        )

                segs = _batch_segments(r0, r1)
                relu_t = []
                for a in range(NA):
                    rt_ = relu_pool.tile([128, RC], BF16, tag=f"relu{a}")
                    for si, (b, c0, c1) in enumerate(segs):
                        if (a + si) % 2 == 0:
                            nc.scalar.activation(
                                rt_[:, c0:c1],
                                pm[a][:, c0:c1],
                                Relu,
                                bias=att2T[:, a, b : b + 1],
                            )
                        else:
                            nc.vector.tensor_scalar(
                                rt_[:, c0:c1],
                                pm[a][:, c0:c1],
                                att2T[:, a, b : b + 1],
                                0.0,
                                mybir.AluOpType.add,
                                mybir.AluOpType.max,
                            )
                    relu_t.append(rt_)

                pa = psAtt.tile([1, RC], F32, tag="psAtt")
                for a in range(NA):
                    nc.tensor.matmul(
                        pa[:, :w],
                        wfull_sb[:, a : a + 1],
                        relu_t[a][:, :w],
                        start=(a == 0),
                        stop=(a == NA - 1),
                    )
                nc.vector.tensor_copy(att_all[:, r0:r1], pa[:, :w])
                nc.sync.dma_start(
                    att_dram[r0:r1].rearrange("(o r) -> o r", o=1),
                    att_all[0:1, r0:r1],
                )
                bdone = r1 // P_PIX  # batches fully bounced
                if bdone > gathered:
                    nc.sync.dma_start(
                        att_bp[gathered:bdone, :],
                        att_dram[gathered * P_PIX : bdone * P_PIX].rearrange(
                            "(b p) -> b p", b=bdone - gathered
                        ),
                    )
                    gathered = bdone

                # fused awe: transpose raw att rows to partitions, exp on
                # 128 lanes, then mask into block-diagonal weight tiles
                wblks = {}
                for t, h, c0 in tiles:
                    ptw = psAtt.tile([128, 1], F32, tag="psAtt")
                    nc.tensor.transpose(
                        ptw[:h, :],
                        att_all[0:1, r0 + c0 : r0 + c0 + h],
                        ident_f[0:1, 0:1],
                    )
                    wexp_t = wexp_pool.tile([128, 1], F32, tag="wexp", name=f"we{t}")
                    nc.scalar.activation(wexp_t[:h, :], ptw[:h, :], Exp, bias=neg8[:h, :])
                    wb = wbp[t - tiles[0][0]]
                    nc.vector.tensor_scalar(
                        wb[:h, :B_LOC],
                        maskt_sb[:h, t, :],
                        wexp_t[:h, 0:1],
                        None,
                        mybir.AluOpType.mult,
                    )
                    wblks[t] = wb

                # V += wblk.T @ enc  (per e-chunk, accumulated over tiles)
                for ec in range(4):
                    pw = psM.tile([128, 512], F32, tag="psM", name=f"pw{ec}")
                    for i, (t, h, c0) in enumerate(tiles):
                        nc.tensor.matmul(
                            pw[:, :],
                            wblks[t][:h, :],
                            nats[t][:h, ec * 512 : (ec + 1) * 512],
                            start=(i == 0),
                            stop=(i == len(tiles) - 1),
                        )
                    if rc == 0:
                        nc.vector.tensor_copy(
                            vsum[:, ec * 512 : (ec + 1) * 512], pw[:B_LOC, :]
                        )
                    else:
                        nc.vector.tensor_add(
                            vsum[:, ec * 512 : (ec + 1) * 512],
                            vsum[:, ec * 512 : (ec + 1) * 512],
                            pw[:B_LOC, :],
                        )

        # ---------------- Phase C: softmax + outputs ----------------------
        with tc.tile_pool(name="cd", bufs=1) as cd:

            if debug:
                nc.sync.dma_start(dbg_att[:], att_all[:])
                nc.sync.dma_start(dbg_v[:], vsum[:])
            mx = cd.tile([B_LOC, 1], F32)
            nc.vector.reduce_max(mx[:], att_bp[:], axis=AX)
            negm = cd.tile([B_LOC, 1], F32)
            nc.vector.tensor_scalar_mul(negm[:], mx[:], -1.0)
            expv = cd.tile([B_LOC, P_PIX], F32)
            ssum = cd.tile([B_LOC, 1], F32)
            nc.scalar.activation(
                expv[:], att_bp[:], Exp, bias=negm[:], accum_out=ssum[:]
            )
            rsum = cd.tile([B_LOC, 1], F32)
            nc.vector.reciprocal(rsum[:], ssum[:])
            alpha_sb = cd.tile([B_LOC, P_PIX], F32)
            nc.vector.tensor_scalar_mul(alpha_sb[:], expv[:], rsum[:])
            nc.sync.dma_start(out_alpha[:], alpha_sb[:])

            # awe = V / Z8;  Z8 = sum(exp(att - ESHIFT)) per batch
            exp8 = cd.tile([B_LOC, P_PIX], F32)
            z8 = cd.tile([B_LOC, 1], F32)
            nc.scalar.activation(
                exp8[:], att_bp[:], Exp, bias=neg8[:B_LOC, :], accum_out=z8[:]
            )
            rz8 = cd.tile([B_LOC, 1], F32)
            nc.vector.reciprocal(rz8[:], z8[:])
            awe_sb = cd.tile([B_LOC, E_DIM], F32)
            nc.vector.tensor_scalar_mul(awe_sb[:], vsum[:], rz8[:])
            nc.sync.dma_start(out_awe[:], awe_sb[:])

    nc.compile()
    return nc


def _make_maskt():
    """maskt[p, t, b] = 1.0 iff global row 128*t + p belongs to batch b."""
    m = np.zeros((128, RT, B_LOC), dtype=np.float32)
    for t in range(RT):
        for p in range(128):
            row = 128 * t + p
            if row < ROWS:
                m[p, t, row // P_PIX] = 1.0
    return m


def _prep_in_maps(inputs):
    import ml_dtypes

    enc = np.ascontiguousarray(np.asarray(inputs["encoder_out"], dtype=np.float32))
    dec = np.ascontiguousarray(
        np.asarray(inputs["decoder_hidden"], dtype=np.float32).astype(ml_dtypes.bfloat16)
    )
    w_enc = np.ascontiguousarray(
        np.asarray(inputs["W_enc"], dtype=np.float32).astype(ml_dtypes.bfloat16)
    )
    w_dec = np.ascontiguousarray(
        np.asarray(inputs["W_dec"], dtype=np.float32).astype(ml_dtypes.bfloat16)
    )
    b_enc = np.asarray(inputs["b_enc"], dtype=np.float32)
    b_dec = np.asarray(inputs["b_dec"], dtype=np.float32)
    w_full = np.ascontiguousarray(
        np.asarray(inputs["W_full"], dtype=np.float32).astype(ml_dtypes.bfloat16)
    )
    bias_a = np.ascontiguousarray(b_enc + b_dec)
    maskt = _make_maskt()

    in_maps = []
    for c in range(N_CORES):
        b0 = c * B_LOC
        in_maps.append(
            {
                "enc": np.ascontiguousarray(
                    enc[b0 : b0 + B_LOC].reshape(ROWS, E_DIM)
                ),
                "dec": np.ascontiguousarray(dec[b0 : b0 + B_LOC]),
                "w_enc": w_enc,
                "w_dec": w_dec,
                "bias_a": bias_a,
                "w_full": w_full,
                "maskt": maskt,
            }
        )
    return in_maps


def _run(inputs, trace=False):
    from concourse.bass_utils import run_bass_kernel_spmd

    if "nc" not in _CACHE:
        _CACHE["nc"] = _build()
    nc = _CACHE["nc"]
    in_maps = _prep_in_maps(inputs)
    res = run_bass_kernel_spmd(nc, in_maps, core_ids=list(range(N_CORES)), trace=trace)
    outs = res.results
    awe = np.concatenate([outs[c]["out_awe"] for c in range(N_CORES)], axis=0)
    alpha = np.concatenate([outs[c]["out_alpha"] for c in range(N_CORES)], axis=0)
    return (awe.astype(np.float32), alpha.astype(np.float32)), res


def kernel(**inputs):
    (awe, alpha), _ = _run(inputs, trace=False)
    return (awe, alpha)


# revision 52
# speedup vs baseline: 1.0176x; 1.0176x over previous
"""Trainium2 Bass kernel for AdditiveVisioLinguistic attention.

reference:
    att1 = enc @ W_enc + b_enc              # [B, P, A]
    att2 = dec @ W_dec + b_dec              # [B, A]
    att  = relu(att1 + att2[:, None]) @ W_full + b_full   # [B, P]
    alpha = softmax(att, axis=1)
    awe  = einsum('bpe,bp->be', enc, alpha)
    return (awe, alpha)

Sharding: pure data parallel over the batch dim (16 batches per core, 8
cores), weights replicated.  b_full is dropped on-device (softmax is shift
invariant); b_enc+b_dec are merged host-side; W_enc/W_dec/W_full/dec are
pre-cast to bf16 host-side (the big matmuls run in bf16, PSUM accumulates
fp32; worst-case rel err ~2.5e-3).

Per-core: rows = 16*196 = 3136 flattened (b,p) rows, processed in
r-tile-aligned chunks (a small 128-row first chunk absorbs the DMA ramp,
then 512-row chunks).  The E=2048 contraction needs enc with E on
partitions: enc tiles are cast to bf16 and PE-transposed (bf16 transposes
may write bf16 PSUM), then copied to SBUF; att1 accumulates over 16
K-chunks into 4 PSUM banks; add-bias+ReLU is fused into one ScalarE
activation per batch segment (bias = att2T column); att comes from a
W_full matvec as [1, rows].

awe is fused into the main loop ("flash" style): per chunk, the raw att
rows are PE-transposed onto partitions, w = exp(att - 8) is computed on
128 lanes and masked into block-diagonal weight tiles (host-provided
mask), and V += wblk.T @ enc runs on the still-resident bf16 enc tiles.
Since the final normalization awe = V / sum(exp(att - 8)) is exactly
shift-invariant, a fixed shift of 8 replaces the running max (att for
this data is O(1), so exp stays well inside fp32 range).  alpha uses a
true max-subtracted softmax on a [16, 196] layout gathered incrementally
through a DRAM bounce (cross-partition reshapes are DMA-only on TRN2).
"""

import numpy as np

N_CORES = 8
B_GLOB = 128
B_LOC = B_GLOB // N_CORES     # 16
P_PIX = 196
E_DIM = 2048
A_DIM = 512
D_DIM = 512

ROWS = B_LOC * P_PIX          # 3136
RC = 512                      # rows per main chunk
N_RC = (ROWS + RC - 1) // RC  # 7 (last chunk has 64 valid rows)
KE = E_DIM // 128             # 16
KD = D_DIM // 128             # 4
NA = A_DIM // 128             # 4
RT = (ROWS + 127) // 128      # 25 r-tiles (last has 64 valid rows)
ESHIFT = 8.0                  # fixed exp shift for the fused awe pass

_CACHE = {}


def _batch_segments(r0, r1):
    """Batch segments overlapping rows [r0, r1): list of (batch, c0, c1)
    where [c0, c1) are column offsets relative to r0."""
    segs = []
    b0 = r0 // P_PIX
    b1 = (min(r1, ROWS) - 1) // P_PIX
    for b in range(b0, b1 + 1):
        s0 = max(r0, b * P_PIX)
        s1 = min(min(r1, ROWS), (b + 1) * P_PIX)
        if s1 > s0:
            segs.append((b, s0 - r0, s1 - r0))
    return segs


def _build(debug=False):
    import concourse.bacc as bacc
    import concourse.mybir as mybir
    import concourse.tile as tile
    from concourse.masks import make_identity
    from contextlib import ExitStack

    F32 = mybir.dt.float32
    F32R = mybir.dt.float32r
    BF16 = mybir.dt.bfloat16
    Relu = mybir.ActivationFunctionType.Relu
    Exp = mybir.ActivationFunctionType.Exp
    AX = mybir.AxisListType.X

    def r_(ap):
        return ap.bitcast(F32R)

    nc = bacc.Bacc(None, target_bir_lowering=False)

    enc = nc.dram_tensor("enc", [ROWS, E_DIM], F32, kind="ExternalInput")
    dec = nc.dram_tensor("dec", [B_LOC, D_DIM], BF16, kind="ExternalInput")
    w_enc = nc.dram_tensor("w_enc", [E_DIM, A_DIM], BF16, kind="ExternalInput")
    w_dec = nc.dram_tensor("w_dec", [D_DIM, A_DIM], BF16, kind="ExternalInput")
    bias_a = nc.dram_tensor("bias_a", [A_DIM], F32, kind="ExternalInput")
    w_full = nc.dram_tensor("w_full", [A_DIM], BF16, kind="ExternalInput")
    maskt = nc.dram_tensor("maskt", [128, RT, B_LOC], F32, kind="ExternalInput")
    att_dram = nc.dram_tensor("att_bounce", [ROWS], F32)
    out_awe = nc.dram_tensor("out_awe", [B_LOC, E_DIM], F32, kind="ExternalOutput")
    out_alpha = nc.dram_tensor(
        "out_alpha", [B_LOC, P_PIX], F32, kind="ExternalOutput"
    )
    if debug:
        dbg_att = nc.dram_tensor("dbg_att", [1, ROWS], F32, kind="ExternalOutput")
        dbg_v = nc.dram_tensor("dbg_v", [B_LOC, E_DIM], F32, kind="ExternalOutput")

    with tile.TileContext(nc) as tc, ExitStack() as ctx:
        consts = ctx.enter_context(tc.tile_pool(name="consts", bufs=1))

        ident_f = consts.tile([128, 128], F32)
        make_identity(nc, ident_f[:])
        ident = consts.tile([128, 128], F32)
        nc.vector.tensor_copy(r_(ident[:]), ident_f[:])
        ident_bf = consts.tile([128, 128], BF16)
        nc.vector.tensor_copy(ident_bf[:], ident_f[:])

        # warm the PE clock gate during the initial DMA wait: dummy
        # transposes of the identity (last one overwrites ident_bf with
        # ident_bf.T == identity, so the chain is not dead code)
        with tc.tile_pool(name="psWarm", bufs=2, space="PSUM") as psWarm:
            wp = None
            for _ in range(36):
                wp = psWarm.tile([128, 128], BF16, tag="warm", name="wp")
                nc.tensor.transpose(wp[:], ident_bf[:], ident_bf[:])
            nc.vector.tensor_copy(ident_bf[:], wp[:])

        dec_sb = consts.tile([B_LOC, D_DIM], BF16)
        nc.sync.dma_start(dec_sb[:], dec[:])
        wdec_sb = consts.tile([128, KD, A_DIM], BF16)
        nc.sync.dma_start(
            wdec_sb[:], w_dec[:].rearrange("(ko ki) a -> ki ko a", ki=128)
        )
        bias_sb = consts.tile([128, NA], F32)
        nc.sync.dma_start(bias_sb[:], bias_a[:].rearrange("(o p) -> p o", p=128))
        wfull_sb = consts.tile([128, NA], BF16)
        nc.sync.dma_start(wfull_sb[:], w_full[:].rearrange("(o p) -> p o", p=128))


        att_all = consts.tile([1, ROWS], F32)
        att_bp = consts.tile([B_LOC, P_PIX], F32)
        neg8 = consts.tile([128, 1], F32)
        nc.vector.memset(neg8[:], -ESHIFT)
        att2T = consts.tile([128, NA, B_LOC], F32)
        vsum = consts.tile([B_LOC, E_DIM], F32)  # unnormalized awe accumulator
        wbp = []
        for i in range(4):
            wt = consts.tile([128, 128], BF16, name=f"wbp{i}")
            nc.vector.memset(wt[:], 0.0)
            wbp.append(wt)

        # ---------------- Phase A: att2T = (dec @ W_dec + bias).T ----------
        with tc.tile_pool(name="psA", bufs=2, space="PSUM") as psA:
            decT = consts.tile([128, KD, B_LOC], BF16)
            for kc in range(KD):
                pt = psA.tile([128, B_LOC], BF16, tag="psA")
                nc.tensor.transpose(
                    pt[:],
                    dec_sb[:, kc * 128 : (kc + 1) * 128],
                    ident_bf[:B_LOC, :B_LOC],
                )
                nc.vector.tensor_copy(decT[:, kc, :], pt[:])
            for a in range(NA):
                pa = psA.tile([128, B_LOC], F32, tag="psA2")
                for kc in range(KD):
                    nc.tensor.matmul(
                        pa[:],
                        wdec_sb[:, kc, a * 128 : (a + 1) * 128],
                        decT[:, kc, :],
                        start=(kc == 0),
                        stop=(kc == KD - 1),
                    )
                nc.vector.tensor_scalar_add(att2T[:, a, :], pa[:], bias_sb[:, a : a + 1])

        # ------------- Phase B: att1, relu, att, fused awe accumulate ------
        with (
            tc.tile_pool(name="nat", bufs=5) as nat_pool,
            tc.tile_pool(name="natbf", bufs=9) as natbf_pool,
            tc.tile_pool(name="encT", bufs=3) as encT_pool,
            tc.tile_pool(name="relu", bufs=2) as relu_pool,
            tc.tile_pool(name="wexp", bufs=2) as wexp_pool,
            tc.tile_pool(name="wblk", bufs=8) as wblk_pool,
            tc.tile_pool(name="psM", bufs=4, space="PSUM") as psM,
            tc.tile_pool(name="psT", bufs=3, space="PSUM") as psT,
            tc.tile_pool(name="psAtt", bufs=1, space="PSUM") as psAtt,
        ):
            # prefetch the first chunk's enc tiles ahead of the big weight DMA
            nats_cache = {}
            for i, t in enumerate(range(0, 4)):
                h = min(128, ROWS - 128 * t)
                nt = nat_pool.tile([128, E_DIM], F32, tag="nat", name=f"pre{t}")
                nc.sync.dma_start(r_(nt[:h, :]), r_(enc[128 * t : 128 * t + h, :]))
                nb = natbf_pool.tile([128, E_DIM], BF16, tag="natbf", name=f"prebf{t}")
                nc.vector.tensor_copy(nb[:h, : E_DIM // 2], nt[:h, : E_DIM // 2])
                nc.scalar.copy(nb[:h, E_DIM // 2 :], nt[:h, E_DIM // 2 :])
                nats_cache[t] = nb
            maskt_sb = consts.tile([128, RT, B_LOC], F32)
            nc.sync.dma_start(maskt_sb[:], maskt[:])
            wenc_sb = consts.tile([128, KE, A_DIM], BF16)
            nc.sync.dma_start(
                wenc_sb[:], w_enc[:].rearrange("(ko ki) a -> ki ko a", ki=128)
            )
            bounds = [0, 128, 640, 1152, 1664, 2176, 2688, ROWS]
            gathered = 0
            for rc in range(len(bounds) - 1):
                r0 = bounds[rc]
                r1 = bounds[rc + 1]
                w = r1 - r0
                tiles = []  # (t, h, col0)
                for t in range(r0 // 128, (r1 + 127) // 128):
                    h = min(128, ROWS - 128 * t)
                    tiles.append((t, h, 128 * t - r0))

                nats = {}
                for i, (t, h, _) in enumerate(tiles):
                    if t in nats_cache:
                        nats[t] = nats_cache.pop(t)
                        continue
                    nt = nat_pool.tile([128, E_DIM], F32, tag="nat")
                    nc.sync.dma_start(r_(nt[:h, :]), r_(enc[128 * t : 128 * t + h, :]))
                    nb = natbf_pool.tile([128, E_DIM], BF16, tag="natbf", name=f"nb{t}")
                    if i % 2 == 0:
                        nc.vector.tensor_copy(nb[:h, :], nt[:h, :])
                    else:
                        nc.scalar.copy(nb[:h, :], nt[:h, :])
                    nats[t] = nb

                enT = encT_pool.tile([128, KE, RC], BF16, tag="encT")
                for k in range(KE):
                    pt = psT.tile([128, RC], BF16, tag="psT")
                    for t, h, c0 in tiles:
                        nc.tensor.transpose(
                            pt[:, c0 : c0 + h],
                            nats[t][:h, k * 128 : (k + 1) * 128],
                            ident_bf[:h, :h],
                        )
                    # alternate copy engine to balance DVE/ACT load
                    if k % 2 == 0:
                        nc.vector.tensor_copy(enT[:, k, :w], pt[:, :w])
                    else:
                        nc.scalar.copy(enT[:, k, :w], pt[:, :w])

                pm = [psM.tile([128, RC], F32, tag="psM", name=f"pm{a}") for a in range(NA)]
                for k in range(KE):
                    for a in range(NA):
                        nc.tensor.matmul(
                            pm[a][:, :w],
                            wenc_sb[:, k, a * 128 : (a + 1) * 128],
                            enT[:, k, :w],
                            start=(k == 0),
                            stop=(k == KE - 1),
                        )

                segs = _batch_segments(r0, r1)
                relu_t = []
                for a in range(NA):
                    rt_ = relu_pool.tile([128, RC], BF16, tag=f"relu{a}")
                    for si, (b, c0, c1) in enumerate(segs):
                        if (a + si) % 2 == 0:
                            nc.scalar.activation(
                                rt_[:, c0:c1],
                                pm[a][:, c0:c1],
                                Relu,
                                bias=att2T[:, a, b : b + 1],
                            )
                        else:
                            nc.vector.tensor_scalar(
                                rt_[:, c0:c1],
                                pm[a][:, c0:c1],
                                att2T[:, a, b : b + 1],
                                0.0,
                                mybir.AluOpType.add,
                                mybir.AluOpType.max,
                            )
                    relu_t.append(rt_)

                pa = psAtt.tile([1, RC], F32, tag="psAtt")
                for a in range(NA):
                    nc.tensor.matmul(
                        pa[:, :w],
                        wfull_sb[:, a : a + 1],
                        relu_t[a][:, :w],
                        start=(a == 0),
                        stop=(a == NA - 1),
                    )
                nc.vector.tensor_copy(att_all[:, r0:r1], pa[:, :w])
                nc.sync.dma_start(
                    att_dram[r0:r1].rearrange("(o r) -> o r", o=1),
                    att_all[0:1, r0:r1],
                )
                bdone = r1 // P_PIX  # batches fully bounced
                if bdone > gathered:
                    nc.sync.dma_start(
                        att_bp[gathered:bdone, :],
                        att_dram[gathered * P_PIX : bdone * P_PIX].rearrange(
                            "(b p) -> b p", b=bdone - gathered
                        ),
                    )
                    gathered = bdone

                # fused awe: transpose raw att rows to partitions, exp on
                # 128 lanes, then mask into block-diagonal weight tiles
                wblks = {}
                for t, h, c0 in tiles:
                    ptw = psAtt.tile([128, 1], F32, tag="psAtt")
                    nc.tensor.transpose(
                        ptw[:h, :],
                        att_all[0:1, r0 + c0 : r0 + c0 + h],
                        ident_f[0:1, 0:1],
                    )
                    wexp_t = wexp_pool.tile([128, 1], F32, tag="wexp", name=f"we{t}")
                    nc.scalar.activation(wexp_t[:h, :], ptw[:h, :], Exp, bias=neg8[:h, :])
                    wb = wbp[t - tiles[0][0]]
                    nc.vector.tensor_scalar(
                        wb[:h, :B_LOC],
                        maskt_sb[:h, t, :],
                        wexp_t[:h, 0:1],
                        None,
                        mybir.AluOpType.mult,
                    )
                    wblks[t] = wb

                # V += wblk.T @ enc  (per e-chunk, accumulated over tiles)
                for ec in range(4):
                    pw = psM.tile([128, 512], F32, tag="psM", name=f"pw{ec}")
                    for i, (t, h, c0) in enumerate(tiles):
                        nc.tensor.matmul(
                            pw[:, :],
                            wblks[t][:h, :],
                            nats[t][:h, ec * 512 : (ec + 1) * 512],
                            start=(i == 0),
                            stop=(i == len(tiles) - 1),
                        )
                    if rc == 0:
                        nc.vector.tensor_copy(
                            vsum[:, ec * 512 : (ec + 1) * 512], pw[:B_LOC, :]
                        )
                    else:
                        nc.vector.tensor_add(
                            vsum[:, ec * 512 : (ec + 1) * 512],
                            vsum[:, ec * 512 : (ec + 1) * 512],
                            pw[:B_LOC, :],
                        )

        # ---------------- Phase C: softmax + outputs ----------------------
        with tc.tile_pool(name="cd", bufs=1) as cd:

            if debug:
                nc.sync.dma_start(dbg_att[:], att_all[:])
                nc.sync.dma_start(dbg_v[:], vsum[:])
            mx = cd.tile([B_LOC, 1], F32)
            nc.vector.reduce_max(mx[:], att_bp[:], axis=AX)
            negm = cd.tile([B_LOC, 1], F32)
            nc.vector.tensor_scalar_mul(negm[:], mx[:], -1.0)
            expv = cd.tile([B_LOC, P_PIX], F32)
            ssum = cd.tile([B_LOC, 1], F32)
            nc.scalar.activation(
                expv[:], att_bp[:], Exp, bias=negm[:], accum_out=ssum[:]
            )
            rsum = cd.tile([B_LOC, 1], F32)
            nc.vector.reciprocal(rsum[:], ssum[:])
            alpha_sb = cd.tile([B_LOC, P_PIX], F32)
            nc.vector.tensor_scalar_mul(alpha_sb[:], expv[:], rsum[:])
            nc.sync.dma_start(out_alpha[:], alpha_sb[:])

            # awe = V / Z8;  Z8 = sum(exp(att - ESHIFT)) per batch
            exp8 = cd.tile([B_LOC, P_PIX], F32)
            z8 = cd.tile([B_LOC, 1], F32)
            nc.scalar.activation(
                exp8[:], att_bp[:], Exp, bias=neg8[:B_LOC, :], accum_out=z8[:]
            )
            rz8 = cd.tile([B_LOC, 1], F32)
            nc.vector.reciprocal(rz8[:], z8[:])
            awe_sb = cd.tile([B_LOC, E_DIM], F32)
            half = E_DIM // 2
            nc.vector.tensor_scalar_mul(awe_sb[:, :half], vsum[:, :half], rz8[:])
            nc.scalar.activation(
                awe_sb[:, half:], vsum[:, half:],
                mybir.ActivationFunctionType.Copy, scale=rz8[:],
            )
            nc.sync.dma_start(out_awe[:], awe_sb[:])

    nc.compile()
    return nc


def _make_maskt():
    """maskt[p, t, b] = 1.0 iff global row 128*t + p belongs to batch b."""
    m = np.zeros((128, RT, B_LOC), dtype=np.float32)
    for t in range(RT):
        for p in range(128):
            row = 128 * t + p
            if row < ROWS:
                m[p, t, row // P_PIX] = 1.0
    return m


def _prep_in_maps(inputs):
    import ml_dtypes

    enc = np.ascontiguousarray(np.asarray(inputs["encoder_out"], dtype=np.float32))
    dec = np.ascontiguousarray(
        np.asarray(inputs["decoder_hidden"], dtype=np.float32).astype(ml_dtypes.bfloat16)
    )
    w_enc = np.ascontiguousarray(
        np.asarray(inputs["W_enc"], dtype=np.float32).astype(ml_dtypes.bfloat16)
    )
    w_dec = np.ascontiguousarray(
        np.asarray(inputs["W_dec"], dtype=np.float32).astype(ml_dtypes.bfloat16)
    )
    b_enc = np.asarray(inputs["b_enc"], dtype=np.float32)
    b_dec = np.asarray(inputs["b_dec"], dtype=np.float32)
    w_full = np.ascontiguousarray(
        np.asarray(inputs["W_full"], dtype=np.float32).astype(ml_dtypes.bfloat16)
    )
    bias_a = np.ascontiguousarray(b_enc + b_dec)
    maskt = _make_maskt()

    in_maps = []
    for c in range(N_CORES):
        b0 = c * B_LOC
        in_maps.append(
            {
                "enc": np.ascontiguousarray(
                    enc[b0 : b0 + B_LOC].reshape(ROWS, E_DIM)
                ),
                "dec": np.ascontiguousarray(dec[b0 : b0 + B_LOC]),
                "w_enc": w_enc,
                "w_dec": w_dec,
                "bias_a": bias_a,
                "w_full": w_full,
                "maskt": maskt,
            }
        )
    return in_maps


def _run(inputs, trace=False):
    from concourse.bass_utils import run_bass_kernel_spmd

    if "nc" not in _CACHE:
        _CACHE["nc"] = _build()
    nc = _CACHE["nc"]
    in_maps = _prep_in_maps(inputs)
    res = run_bass_kernel_spmd(nc, in_maps, core_ids=list(range(N_CORES)), trace=trace)
    outs = res.results
    awe = np.concatenate([outs[c]["out_awe"] for c in range(N_CORES)], axis=0)
    alpha = np.concatenate([outs[c]["out_alpha"] for c in range(N_CORES)], axis=0)
    return (awe.astype(np.float32), alpha.astype(np.float32)), res


def kernel(**inputs):
    (awe, alpha), _ = _run(inputs, trace=False)
    return (awe, alpha)
